# revision 5
# baseline (speedup 1.0000x reference)
"""Trainium2 Bass kernel for nn_LoadPathLoss.

reference computation:
  structure = state[:, ch]                  # [B=4, D=64, H=128, W=128]
  s = structure[:, 0]
  for z in 1..63:  s = max(s, min(structure[:, z], maxpool3x3(s)))
  return relu(structure - s[:, None]).mean()

Data parallel over B=4 with the z-scan replicated on core pairs (cores b and
b+4 both scan batch b; the final-phase work is split between them).  All
on-chip compute is float16: values are shifted +8 on the host so the PE
shift-matmuls' zero boundary rows act as -inf while staying in fp16's
high-precision range (scan error ~1e-4 absolute vs the 2e-2 rel tolerance).

Per z-step (serial 63-step chain; ~1.69us/step):
  - three fp16 matmuls (up-shift, down-shift, identity lhsT, built on-chip
    with gpsimd.affine_select) write [S_up, S_dn, S] into one PSUM tile;
  - m = max(S, c_z) on DVE, hidden under the PE->PSUM latency;
  - colmax = max of the three PSUM planes in ONE strided tensor_reduce
    (a single PSUM access pattern, sidestepping the one-PSUM-input rule);
  - rowmax3 via two fp16 tensor_tensor ops on the zero-padded colmax tile;
  - S = min(rowmax, m).
Phase 2 (chunks split across the core pair): chunks[k] = max(chunks[k], s)
in place (fp16 2x), PE ones-matmul column sums accumulate into one PSUM
bank, Activation engine copies PSUM->SBUF, one output DMA.  Host combines
in f64:  mean = [sum max(c,s) - D * sum s] / (B*D*H*W)  (the +8 cancels).
"""

import numpy as np

B, C, D, H, W = 4, 8, 64, 128, 128
NCORES = 4
ZCHUNK = 8          # z-slices per DMA chunk / phase-2 op
NCHUNK = D // ZCHUNK
SHIFT = 8.0
_cached = {}


def _build_nc(d_steps=D, p2_chunks=None, with_ssum=True):
    import concourse.bacc as bacc
    import concourse.mybir as mybir
    from concourse.tile import TileContext

    fp32 = mybir.dt.float32
    fp16 = mybir.dt.float16
    mx = mybir.AluOpType.max
    mn = mybir.AluOpType.min

    nc = bacc.Bacc("TRN2", target_bir_lowering=False, debug=False)
    cb = nc.dram_tensor("cb", [D, H, W], fp16, kind="ExternalInput")
    out = nc.dram_tensor("out", [1, 512 + W], fp32,
                         kind="ExternalOutput")  # [1, 640]

    HALF = 512              # columns per PSUM accum bank

    with TileContext(nc) as tc:
        with (
            tc.tile_pool(name="sbuf", bufs=1) as pool,
            tc.tile_pool(name="psum", bufs=3, space="PSUM") as psum,
            tc.tile_pool(name="psacc", bufs=1, space="PSUM") as psacc,
        ):
            sh = pool.tile([H, 3 * H + 1], fp16, tag="sh")
            ones_hh = pool.tile([H, H], fp16, tag="ones_hh")
            chunks = [
                pool.tile([H, ZCHUNK, W], fp16, tag=f"cb{k}", name=f"cb{k}")
                for k in range(NCHUNK)
            ]
            c00 = pool.tile([H, 2, W], fp16, tag="c00")
            S = pool.tile([H, W], fp16, tag="S")
            cm = pool.tile([H, W + 2], fp16, tag="cm")      # zero-padded cols
            q = pool.tile([H, W + 1], fp16, tag="q")
            below = pool.tile([H, W], fp16, tag="below")
            m = pool.tile([H, W], fp16, tag="m")

            # tiny two-slice DMA so steps 0-1 start before chunk 0 lands
            nc.sync.dma_start(out=c00[:], in_=cb[0:2].rearrange("z h w -> h z w"))
            # shift matrices built on-chip (no weights DMA):
            #   sh[:, 0:H] = U (out[p] = x[p+1]):   1 where col == row-1
            #   sh[:, H:2H] = D (out[p] = x[p-1]):  1 where col == row+1
            #   sh[:, 2H:3H] = I;  sh[:, 3H] = ones
            nc.vector.memset(ones_hh[:], 1.0)
            eq = mybir.AluOpType.is_equal
            nc.gpsimd.affine_select(sh[:, 0:H], ones_hh[:], [[-1, H]], eq, 0.0,
                                    base=-1, channel_multiplier=1)
            nc.gpsimd.affine_select(sh[:, H : 2 * H], ones_hh[:], [[-1, H]], eq,
                                    0.0, base=1, channel_multiplier=1)
            nc.gpsimd.affine_select(sh[:, 2 * H : 3 * H], ones_hh[:], [[-1, H]],
                                    eq, 0.0, base=0, channel_multiplier=1)
            nc.vector.memset(sh[:, 3 * H : 3 * H + 1], 1.0)
            for k in range(NCHUNK):
                src = cb[k * ZCHUNK : (k + 1) * ZCHUNK].rearrange("z h w -> h z w")
                nc.sync.dma_start(out=chunks[k][:], in_=src)

            # zero-pad border columns of cm once; center overwritten each step
            nc.vector.memset(cm[:], 0.0)

            # S = c_0 (already shifted +8 on host)
            nc.vector.tensor_copy(S[:], c00[:, 0, :])

            for z in range(1, d_steps):
                k, j = z // ZCHUNK, z % ZCHUNK
                c_z = c00[:, 1, :] if z == 1 else chunks[k][:, j, :]
                ps = psum.tile([H, 3, W], fp32, tag="ps", name=f"ps{z}")
                nc.tensor.matmul(
                    out=ps[:, 0, :], lhsT=sh[:, 2 * H : 3 * H], rhs=S[:],
                    start=True, stop=True,
                )
                nc.tensor.matmul(
                    out=ps[:, 1, :], lhsT=sh[:, 0:H], rhs=S[:],
                    start=True, stop=True,
                )
                nc.tensor.matmul(
                    out=ps[:, 2, :], lhsT=sh[:, H : 2 * H], rhs=S[:],
                    start=True, stop=True,
                )
                # m = max(S, c_z) on DVE, hidden under the PE/PSUM latency
                nc.vector.tensor_tensor(out=m[:], in0=S[:], in1=c_z, op=mx)
                # colmax = max(S, up, dn) in ONE psum read: 3-way strided
                # tensor_reduce over the [H, W, 3] transposed view
                nc.vector.tensor_reduce(
                    out=cm[:, 1 : W + 1], in_=ps[:].transpose([0, 2, 1]),
                    axis=mybir.AxisListType.X, op=mx,
                )
                nc.vector.tensor_tensor(
                    out=q[:, 0 : W + 1], in0=cm[:, 0 : W + 1],
                    in1=cm[:, 1 : W + 2], op=mx,
                )
                nc.vector.tensor_tensor(
                    out=below[:], in0=q[:, 0:W], in1=cm[:, 2 : W + 2], op=mx
                )
                nc.vector.tensor_tensor(
                    out=S[:], in0=below[:], in1=m[:], op=mn
                )

            # ---- phase 2 ----
            # per chunk: chunks[k] = max(chunks[k], s) in place (fp16, 2x),
            # then PE ones-matmul column sums accumulate into one PSUM bank.
            # Each core pair splits the chunks (the scan ran redundantly on
            # both); PSUM->SBUF copies run on the idle Activation engine.
            if p2_chunks is None:
                p2_chunks = list(range(NCHUNK))
            sbc = S[:].unsqueeze(1).broadcast_to((H, ZCHUNK, W))
            ones_col = sh[:, 3 * H : 3 * H + 1]  # [H, 1] of ones
            acc_a = psacc.tile([1, HALF], fp32, tag="acc_a")
            acc_s = (psacc.tile([1, W], fp32, tag="acc_s", name="acc_s")
                     if with_ssum else None)
            accs = pool.tile([1, HALF + W], fp32, tag="accs")

            if with_ssum:
                nc.tensor.matmul(out=acc_s[:], lhsT=ones_col, rhs=S[:],
                                 start=True, stop=True)
                nc.scalar.copy(accs[:, HALF : HALF + W], acc_s[:])
            for k in p2_chunks:
                nc.vector.tensor_tensor(
                    out=chunks[k][:], in0=chunks[k][:], in1=sbc, op=mx
                )
            NSL = ZCHUNK * W // HALF  # matmul slices per chunk
            for i, k in enumerate(p2_chunks):
                flat = chunks[k][:].rearrange("h z w -> h (z w)")
                for j in range(NSL):
                    nc.tensor.matmul(
                        out=acc_a[:], lhsT=ones_col,
                        rhs=flat[:, j * HALF : (j + 1) * HALF],
                        start=(i == 0 and j == 0),
                        stop=(i == len(p2_chunks) - 1 and j == NSL - 1),
                    )
            nc.scalar.copy(accs[:, 0:HALF], acc_a[:])
            if with_ssum:
                nc.sync.dma_start(out=out[:, :], in_=accs[:])
            else:
                nc.sync.dma_start(out=out[:, 0:HALF], in_=accs[:, 0:HALF])

    nc.compile()
    return nc


def _make_runner(nc, dev_lo=0):
    """Cached multi-core PJRT runner (mirrors bass2jax.run_bass_via_pjrt but
    keeps the jitted shard_map so repeat calls skip retrace/recompile)."""
    import jax
    from jax.sharding import Mesh, PartitionSpec
    from jax.experimental.shard_map import shard_map
    import concourse.mybir as mybir
    from concourse import bass2jax

    bass2jax.install_neuronx_cc_hook()

    partition_name = nc.partition_id_tensor.name if nc.partition_id_tensor else None
    in_names, out_names, out_avals, zero_outs = [], [], [], []
    for alloc in nc.m.functions[0].allocations:
        if not isinstance(alloc, mybir.MemoryLocationSet):
            continue
        name = alloc.memorylocations[0].name
        if alloc.kind == "ExternalInput":
            if name != partition_name:
                in_names.append(name)
        elif alloc.kind == "ExternalOutput":
            shape = tuple(alloc.tensor_shape)
            dtype = mybir.dt.np(alloc.dtype)
            out_names.append(name)
            out_avals.append(jax.core.ShapedArray(shape, dtype))
            zero_outs.append(np.zeros(shape, dtype))
    n_params = len(in_names)
    n_outs = len(out_avals)
    all_names = in_names + out_names
    donate = tuple(range(n_params, n_params + n_outs))

    def _body(*args):
        operands = list(args)
        if partition_name is not None:
            operands.append(bass2jax.partition_id_tensor())
        outs = bass2jax._bass_exec_p.bind(
            *operands,
            out_avals=tuple(out_avals),
            in_names=tuple(all_names + ([partition_name] if partition_name else [])),
            out_names=tuple(out_names),
            lowering_input_output_aliases=(),
            sim_require_finite=True,
            sim_require_nnan=True,
            nc=nc,
        )
        return tuple(outs)

    devices = jax.devices()[dev_lo : dev_lo + NCORES]
    mesh = Mesh(np.asarray(devices), ("core",))
    in_specs = (PartitionSpec("core"),) * (n_params + n_outs)
    out_specs = (PartitionSpec("core"),) * n_outs
    sharded = jax.jit(
        shard_map(_body, mesh=mesh, in_specs=in_specs, out_specs=out_specs,
                  check_rep=False),
        donate_argnums=donate, keep_unused=True,
    )

    def run(in_maps):
        args = [
            np.concatenate([np.asarray(m[name]) for m in in_maps], axis=0)
            for name in in_names
        ]
        zouts = [np.concatenate([z] * NCORES, axis=0) for z in zero_outs]
        outs = sharded(*args, *zouts)
        res = []
        for b in range(NCORES):
            d = {}
            for i, name in enumerate(out_names):
                full = np.asarray(outs[i])
                per = full.shape[0] // NCORES
                d[name] = full[b * per : (b + 1) * per]
            res.append(d)
        return res

    return run


def kernel(state, ch_structure):
    if "nc" not in _cached:
        half = NCHUNK // 2
        _cached["nc"] = _build_nc(p2_chunks=list(range(half)), with_ssum=True)
        _cached["nc_b"] = _build_nc(p2_chunks=list(range(half, NCHUNK)),
                                    with_ssum=False)
        _cached["run"] = _make_runner(_cached["nc"], dev_lo=0)
        _cached["run_b"] = _make_runner(_cached["nc_b"], dev_lo=NCORES)

    structure = (
        np.ascontiguousarray(state[:, int(ch_structure)], dtype=np.float32) + SHIFT
    ).astype(np.float16)
    in_maps = [{"cb": structure[b]} for b in range(NCORES)]
    results = _cached["run"](in_maps)
    results_b = _cached["run_b"](in_maps)
    _cached["last"] = (results, results_b)

    total = 0.0
    for b in range(NCORES):
        o = results[b]["out"].astype(np.float64)
        ob = results_b[b]["out"].astype(np.float64)
        total += o[0, :512].sum() + ob[0, :512].sum() \
            - float(D) * o[0, 512:].sum()
    mean = total / float(B * D * H * W)
    return np.asarray(mean, dtype=np.float32)


if __name__ == "__main__":
    rng = np.random.default_rng(0)
    st = rng.standard_normal((B, C, D, H, W)).astype(np.float32)
    print(kernel(st, 3))
